# revision 14
# baseline (speedup 1.0000x reference)
"""AdaAttention Trainium2 kernel — data-parallel over batch across 8 NeuronCores.

Full shapes: h [1024,512], sentinel [1024,512], att_feats [1024,96,2048] -> out [1024,512].
Per core: b=128 batch rows; 24 chunks of 4 slots (512 tokens).

v4 pipeline (fp8 DoubleRow MM1, natural-layout MM2, DVE logits, streaming flash cHat):
  out = tanh(cHat @ W_oT + h @ W_oT + b_o), cHat = sum_s alpha_s * img_all_s.
  C = sum_s exp(l_s) * (img_all_s @ W_oT) accumulates in one persistent PSUM bank
  (no max subtraction: |logits| <~ 12, exp safe in f32); divide by d = sum exp at end.

Per chunk:
  att_feats --1 SWDGE DMA, f32->fp8e4 cast--> nat[b,4,2048]
  4x xbar (u16 view) -> attf[p=g%128, gt, i_s, b]  (g = f//2: fp8 byte pairs)
  MM1 (DoubleRow fp8, W_ae x256): attT[r,x] = relu(psum/256 + b_ae)  bf16
  MM2' (x4 slots): psum[b,a] = attT_i.T @ W_cT (stationary attT blocks)
  hA = tanh(psum + h_e + b_c + b_h) [DVE add + ACT tanh]
  logit col = DVE mult+reduce(hA * wal_rep)   (no PE matmul for logits)
  Z_i[b,o] = attT_i.T @ W_oT -> bf16 ; exp (ACT) -> e_sb[:, 1+4c..]
  flash (deferred 1 chunk): C += diag(e_t) @ Z_t  (diag built on GpSimd)
Final: out = tanh(C/d + h@W_oT + b_o). b_al skipped (softmax shift-invariant).
Startup: chunk-0 cast DMA issued first; prep split so the sentinel cross-engine
chain never head-blocks mm1(0) in the PE FIFO.
"""
import sys

for p in ("/opt/trn_rl_repo", "/opt/pypackages"):
    if p not in sys.path:
        sys.path.insert(0, p)

import numpy as np
import ml_dtypes
from contextlib import ExitStack

import concourse.bass as bass
import concourse.bacc as bacc
import concourse.mybir as mybir
from concourse import tile

F32 = mybir.dt.float32
BF16 = mybir.dt.bfloat16
FP8 = mybir.dt.float8e4
U16 = mybir.dt.uint16
AF = mybir.ActivationFunctionType
ALU = mybir.AluOpType
DR = mybir.MatmulPerfMode.DoubleRow

NCORES = 8
B_LOC = 128          # batch rows per core
S = 96               # attention slots
F = 2048             # att feature size
R = 512              # rnn size
A = 512              # att hidden size
XCHUNK = 512         # tokens per pipeline chunk (4 s-tiles)
NCHUNKS = (B_LOC * S) // XCHUNK   # 24
S_PER_CHUNK = XCHUNK // B_LOC     # 4
GT = F // 256        # 8 double-row f-tiles (256 f's each)
RT = R // 128        # 4
AT = A // 128        # 4
WSCALE = 256.0       # fp8 weight scale for W_ae


def build_nc():
    nc = bacc.Bacc("TRN2", target_bir_lowering=False, debug=False)

    # ---- DRAM parameters (per-core shard shapes) ----
    att_feats = nc.declare_dram_parameter("att_feats", [B_LOC, S, F], F32, isOutput=False)
    h_in = nc.declare_dram_parameter("h", [B_LOC, R], F32, isOutput=False)
    sent_in = nc.declare_dram_parameter("sentinel", [B_LOC, R], F32, isOutput=False)
    # w_ae_dr[p, gt, i, r] = (W_ae*256).T[f, r], f = 2*(gt*128+p)+i   (fp8)
    w_ae_d = nc.declare_dram_parameter("w_ae_dr", [128, GT, 2, R], FP8, isOutput=False)
    w_c_t = nc.declare_dram_parameter("w_c_t", [128, RT, A], BF16, isOutput=False)
    w_s_t = nc.declare_dram_parameter("w_s_t", [128, RT, A], BF16, isOutput=False)
    w_h_t = nc.declare_dram_parameter("w_h_t", [128, RT, A], BF16, isOutput=False)
    w_o_t = nc.declare_dram_parameter("w_o_t", [128, RT, R], BF16, isOutput=False)
    wal_rep_d = nc.declare_dram_parameter("wal_rep", [128, A], BF16, isOutput=False)
    b_ae_d = nc.declare_dram_parameter("b_ae", [128, RT], F32, isOutput=False)
    bcbh_d = nc.declare_dram_parameter("bcbh", [128, A], F32, isOutput=False)   # b_c + b_h
    bsbh_d = nc.declare_dram_parameter("bsbh", [128, A], F32, isOutput=False)   # b_s + b_h
    b_o_bc_d = nc.declare_dram_parameter("b_o_bcast", [128, R], F32, isOutput=False)
    ident_d = nc.declare_dram_parameter("ident", [128, 128], BF16, isOutput=False)
    ident4_d = nc.declare_dram_parameter("ident4", [128, S_PER_CHUNK, 128], BF16, isOutput=False)
    out_d = nc.declare_dram_parameter("out", [B_LOC, R], F32, isOutput=True)

    with tile.TileContext(nc) as tc, ExitStack() as ctx:
        # ---- pools ----
        cp = ctx.enter_context(tc.tile_pool(name="consts", bufs=1))
        nat_p = ctx.enter_context(tc.tile_pool(name="nat", bufs=3))
        stg_p = ctx.enter_context(tc.tile_pool(name="stg", bufs=3))
        attf_p = ctx.enter_context(tc.tile_pool(name="attf", bufs=2))
        attT_p = ctx.enter_context(tc.tile_pool(name="attT", bufs=3))
        hat_p = ctx.enter_context(tc.tile_pool(name="hat", bufs=6))
        z_p = ctx.enter_context(tc.tile_pool(name="zt", bufs=3))
        small_p = ctx.enter_context(tc.tile_pool(name="small", bufs=6))
        msel_p = ctx.enter_context(tc.tile_pool(name="msel", bufs=4))
        lcol_p = ctx.enter_context(tc.tile_pool(name="lcol", bufs=3))
        soft_p = ctx.enter_context(tc.tile_pool(name="soft", bufs=1))
        ps_mm1 = ctx.enter_context(tc.tile_pool(name="ps_mm1", bufs=2, space="PSUM"))
        ps_mm2 = ctx.enter_context(tc.tile_pool(name="ps_mm2", bufs=3, space="PSUM"))
        ps_z = ctx.enter_context(tc.tile_pool(name="ps_z", bufs=2, space="PSUM"))
        ps_chat = ctx.enter_context(tc.tile_pool(name="ps_chat", bufs=1, space="PSUM"))

        nat_tiles = {}

        def stage_in(c):
            s0 = c * S_PER_CHUNK
            nat = nat_p.tile([B_LOC, S_PER_CHUNK, F], FP8, tag="nat", name=f"nat_{c}")
            nc.gpsimd.dma_start(out=nat[:, 0:2, :], in_=att_feats[:, s0:s0 + 2, :])
            stg = stg_p.tile([B_LOC, 2, F], F32, tag="stg", name=f"stg_{c}")
            nc.scalar.dma_start(out=stg[:], in_=att_feats[:, s0 + 2:s0 + 4, :])
            nc.vector.tensor_copy(nat[:, 2, :], stg[:, 0, :])
            nc.scalar.activation(nat[:, 3, :], stg[:, 1, :], AF.Copy)
            nat_tiles[c] = nat

        def const_tile(name, shape, dtype, src):
            t = cp.tile(shape, dtype, tag=name, name=name)
            nc.scalar.dma_start(out=t[:], in_=src[:])
            return t

        # chunk 0 feed first; h / sentinel casts next (tiny; unblock prep)
        stage_in(0)
        h_bf = cp.tile([B_LOC, R], BF16, tag="h_bf", name="h_bf")
        nc.gpsimd.dma_start(out=h_bf[:], in_=h_in[:])
        sent_bf = cp.tile([B_LOC, R], BF16, tag="sent_bf", name="sent_bf")
        nc.gpsimd.dma_start(out=sent_bf[:], in_=sent_in[:])

        # ---- constants (mm1 deps first, then prep deps, then stage_b1 deps) ----
        ident = const_tile("ident", [128, 128], BF16, ident_d)
        ident4 = const_tile("ident4", [128, S_PER_CHUNK, 128], BF16, ident4_d)
        w_ae = const_tile("w_ae", [128, GT, 2, R], FP8, w_ae_d)
        b_ae = const_tile("b_ae", [128, RT], F32, b_ae_d)
        w_h = const_tile("w_h", [128, RT, A], BF16, w_h_t)
        w_s = const_tile("w_s", [128, RT, A], BF16, w_s_t)
        w_o = const_tile("w_o", [128, RT, R], BF16, w_o_t)
        wal_rep = const_tile("wal_rep", [128, A], BF16, wal_rep_d)
        bcbh = const_tile("bcbh", [128, A], F32, bcbh_d)
        bsbh = const_tile("bsbh", [128, A], F32, bsbh_d)
        b_o_bc = const_tile("b_o_bc", [128, R], F32, b_o_bc_d)
        w_c = const_tile("w_c", [128, RT, A], BF16, w_c_t)

        stage_in(1)

        # e_sb[:, t] = exp(logit_t), t=0 sentinel, t=1.. att slots
        e_sb = cp.tile([B_LOC, 1 + S], F32, tag="e_sb", name="e_sb")
        prep_out = {}

        def prep_a():
            """PE-heavy prep + sentinel logit chain (no PE work after the chain)."""
            hT = cp.tile([128, RT, B_LOC], BF16, tag="hT", name="hT")
            sentT = cp.tile([128, RT, B_LOC], BF16, tag="sentT", name="sentT")
            for rb in range(RT):
                pt = ps_mm2.tile([128, 1024], BF16, tag="mm2", name=f"pt_h{rb}")
                nc.tensor.transpose(pt[:, :128], h_bf[:, rb * 128:(rb + 1) * 128], ident[:])
                nc.vector.tensor_copy(hT[:, rb, :], pt[:, :128])
                pt2 = ps_mm2.tile([128, 1024], BF16, tag="mm2", name=f"pt_s{rb}")
                nc.tensor.transpose(pt2[:, :128], sent_bf[:, rb * 128:(rb + 1) * 128], ident[:])
                nc.vector.tensor_copy(sentT[:, rb, :], pt2[:, :128])

            # h_e (natural [b, a]) -> he_c = h_e + b_c + b_h ; he_s = h_e + b_s + b_h
            ps_he = ps_mm2.tile([128, A], F32, tag="mm2", name="ps_he")
            for rb in range(RT):
                nc.tensor.matmul(ps_he[:], hT[:, rb, :], w_h[:, rb, :],
                                 start=(rb == 0), stop=(rb == RT - 1))
            he_c = cp.tile([B_LOC, A], F32, tag="he_c", name="he_c")
            nc.vector.tensor_tensor(out=he_c[:], in0=ps_he[:], in1=bcbh[:], op=ALU.add)
            he_s = cp.tile([B_LOC, A], F32, tag="he_s", name="he_s")
            nc.vector.tensor_tensor(out=he_s[:], in0=ps_he[:], in1=bsbh[:], op=ALU.add)

            # sentinel embed + Zsent + H_o  (PE)
            ps_se = ps_mm2.tile([128, A], F32, tag="mm2", name="ps_se")
            for rb in range(RT):
                nc.tensor.matmul(ps_se[:], sentT[:, rb, :], w_s[:, rb, :],
                                 start=(rb == 0), stop=(rb == RT - 1))
            ps_zs = ps_z.tile([128, R], F32, tag="z", name="ps_zs")
            for rb in range(RT):
                nc.tensor.matmul(ps_zs[:], sentT[:, rb, :], w_o[:, rb, :],
                                 start=(rb == 0), stop=(rb == RT - 1))
            zs_sb = cp.tile([B_LOC, R], BF16, tag="zs_sb", name="zs_sb")
            nc.scalar.activation(zs_sb[:], ps_zs[:], AF.Copy)
            ps_ho = ps_z.tile([128, R], F32, tag="z", name="ps_ho")
            for rb in range(RT):
                nc.tensor.matmul(ps_ho[:], hT[:, rb, :], w_o[:, rb, :],
                                 start=(rb == 0), stop=(rb == RT - 1))
            h_o = cp.tile([B_LOC, R], F32, tag="h_o", name="h_o")
            nc.vector.tensor_tensor(out=h_o[:], in0=ps_ho[:], in1=b_o_bc[:], op=ALU.add)

            # sentinel logit chain (DVE/ACT only)
            pre0 = small_p.tile([B_LOC, A], BF16, tag="hatmp", name="pre0")
            nc.vector.tensor_tensor(out=pre0[:], in0=ps_se[:], in1=he_s[:], op=ALU.add)
            hA0 = hat_p.tile([B_LOC, A], BF16, tag="hat", name="hA0")
            nc.scalar.activation(hA0[:], pre0[:], AF.Tanh)
            ttr0 = small_p.tile([B_LOC, A], BF16, tag="ttro", name="ttr0")
            lc0 = lcol_p.tile([B_LOC, 1], F32, tag="lc", name="lc0")
            nc.vector.tensor_tensor(out=ttr0[:], in0=hA0[:], in1=wal_rep[:], op=ALU.mult)
            nc.vector.tensor_reduce(out=lc0[:], in_=ttr0[:], op=ALU.add,
                                    axis=mybir.AxisListType.X)
            nc.scalar.activation(e_sb[:, 0:1], lc0[:], AF.Exp)
            prep_out.update(h_o=h_o, he_c=he_c, zs_sb=zs_sb)

        def prep_b():
            """Open the persistent cHat accumulation with the sentinel term."""
            ps_cH = ps_chat.tile([B_LOC, R], F32, name="ps_cH")
            ms0 = msel_p.tile([128, 128], BF16, tag="msel", name="ms0")
            nc.vector.tensor_scalar(out=ms0[:], in0=ident[:], scalar1=e_sb[:, 0:1],
                                    scalar2=None, op0=ALU.mult)
            nc.tensor.matmul(ps_cH[:], ms0[:], prep_out["zs_sb"][:], start=True,
                             stop=False, skip_group_check=True)
            prep_out.update(ps_cH=ps_cH)

        # ---- main pipeline stages ----
        attT_chunks = {}
        z_chunks = {}

        def stage_mm1(c):
            nat = nat_tiles.pop(c)
            # 4 per-slot xbar transposes (u16 = fp8 byte pair): attf[p, gt, i_s, b]
            attf = attf_p.tile([128, GT, S_PER_CHUNK, 128], U16, tag="attf", name=f"attf_{c}")
            for i in range(S_PER_CHUNK):
                nc.sync.dma_start(out=attf[:, :, i, :], in_=nat[:, i, :].bitcast(U16),
                                  transpose=True)
            attT = attT_p.tile([128, RT, XCHUNK], BF16, tag="attT", name=f"attT_{c}")
            for rb in range(RT):
                ps1 = ps_mm1.tile([128, XCHUNK], F32, tag="mm1", name=f"ps1_{c}_{rb}")
                for gt in range(GT):
                    rhs = attf[:, gt].bitcast(FP8).rearrange("p s (n two) -> p two s n", two=2)
                    nc.tensor.matmul(ps1[:], w_ae[:, gt, :, rb * 128:(rb + 1) * 128],
                                     rhs, start=(gt == 0), stop=(gt == GT - 1),
                                     perf_mode=DR)
                nc.scalar.activation(attT[:, rb, :], ps1[:], AF.Relu,
                                     bias=b_ae[:, rb:rb + 1], scale=1.0 / WSCALE)
            attT_chunks[c] = attT

        def stage_b1(c):
            attT = attT_chunks.pop(c)
            he_c = prep_out["he_c"]
            zt = z_p.tile([128, S_PER_CHUNK, R], BF16, tag="zt", name=f"zt_{c}")
            lcol = lcol_p.tile([B_LOC, S_PER_CHUNK], F32, tag="lc", name=f"lcol_{c}")
            # PE: all MM2 groups first
            ps2s = []
            for i in range(S_PER_CHUNK):
                ps2 = ps_mm2.tile([128, A], F32, tag="mm2", name=f"ps2_{c}_{i}")
                for rb in range(RT):
                    nc.tensor.matmul(ps2[:], attT[:, rb, i * 128:(i + 1) * 128],
                                     w_c[:, rb, :], start=(rb == 0), stop=(rb == RT - 1))
                ps2s.append(ps2)
            # DVE adds chase the MM2 groups; ACT tanh chases the adds
            tmps = []
            for i in range(S_PER_CHUNK):
                tmp = small_p.tile([B_LOC, A], BF16, tag="hatmp", name=f"hatmp_{c}_{i}")
                nc.vector.tensor_tensor(out=tmp[:], in0=ps2s[i][:], in1=he_c[:], op=ALU.add)
                tmps.append(tmp)
            hts = []
            for i in range(S_PER_CHUNK):
                ht = hat_p.tile([B_LOC, A], BF16, tag="hat", name=f"hat_{c}_{i}")
                nc.scalar.activation(ht[:], tmps[i][:], AF.Tanh)
                hts.append(ht)
            # PE: Z groups
            pszs = []
            for i in range(S_PER_CHUNK):
                psz = ps_z.tile([128, R], F32, tag="z", name=f"psz_{c}_{i}")
                for rb in range(RT):
                    nc.tensor.matmul(psz[:], attT[:, rb, i * 128:(i + 1) * 128],
                                     w_o[:, rb, :], start=(rb == 0), stop=(rb == RT - 1))
                pszs.append(psz)
            # Z copies split DVE/ACT
            for i in range(S_PER_CHUNK):
                if i % 2 == 0:
                    nc.vector.tensor_copy(zt[:, i, :], pszs[i][:])
                else:
                    nc.scalar.activation(zt[:, i, :], pszs[i][:], AF.Copy)
            # logits: DVE mult + reduce per slot, then one exp (ACT)
            for i in range(S_PER_CHUNK):
                ttro = small_p.tile([B_LOC, A], BF16, tag="ttro", name=f"ttro_{c}_{i}")
                nc.vector.tensor_tensor(out=ttro[:], in0=hts[i][:], in1=wal_rep[:],
                                        op=ALU.mult)
                nc.vector.tensor_reduce(out=lcol[:, i:i + 1], in_=ttro[:], op=ALU.add,
                                        axis=mybir.AxisListType.X)
            z_chunks[c] = zt
            nc.scalar.activation(
                e_sb[:, 1 + c * S_PER_CHUNK: 1 + (c + 1) * S_PER_CHUNK], lcol[:], AF.Exp)

        def stage_flash(c):
            ps_cH = prep_out["ps_cH"]
            zt = z_chunks.pop(c)
            ms4 = msel_p.tile([128, S_PER_CHUNK, 128], BF16, tag="msel", name=f"ms4_{c}")
            nc.vector.tensor_tensor(
                out=ms4[:], in0=ident4[:],
                in1=e_sb[:, 1 + c * S_PER_CHUNK: 1 + (c + 1) * S_PER_CHUNK]
                    .unsqueeze(2).broadcast_to([128, S_PER_CHUNK, 128]),
                op=ALU.mult)
            for i in range(S_PER_CHUNK):
                t = c * S_PER_CHUNK + i
                nc.tensor.matmul(ps_cH[:], ms4[:, i, :], zt[:, i, :],
                                 start=False, stop=(t == S - 1), skip_group_check=True)

        # ---- build pipeline ----
        stage_mm1(0)
        prep_a()
        stage_in(2)
        stage_mm1(1)
        stage_in(3)
        prep_b()
        for c in range(2, NCHUNKS + 5):
            if 3 <= c <= NCHUNKS + 2:
                stage_flash(c - 3)
            if 2 <= c <= NCHUNKS + 1:
                stage_b1(c - 2)
            if c < NCHUNKS:
                stage_mm1(c)
            if c + 2 < NCHUNKS:
                stage_in(c + 2)

        # ---- final: out = tanh(C/d + H_o) ----
        ps_cH = prep_out["ps_cH"]
        h_o = prep_out["h_o"]
        dsum = soft_p.tile([B_LOC, 1], F32, tag="soft", name="dsum")
        nc.vector.tensor_reduce(out=dsum[:], in_=e_sb[:], op=ALU.add,
                                axis=mybir.AxisListType.X)
        rin = soft_p.tile([B_LOC, 1], F32, tag="rin", name="rin")
        nc.vector.reciprocal(rin[:], dsum[:])
        chn = soft_p.tile([B_LOC, R], F32, tag="chn", name="chn")
        nc.vector.tensor_scalar(out=chn[:], in0=ps_cH[:], scalar1=rin[:],
                                scalar2=None, op0=ALU.mult)
        pre = soft_p.tile([B_LOC, R], F32, tag="pre", name="pre")
        nc.vector.tensor_tensor(out=pre[:], in0=chn[:], in1=h_o[:], op=ALU.add)
        out_sb = soft_p.tile([B_LOC, R], F32, tag="out_sb", name="out_sb")
        nc.scalar.activation(out_sb[:], pre[:], AF.Tanh)
        nc.sync.dma_start(out=out_d[:], in_=out_sb[:])

    nc.compile()
    return nc


# ---------------- host side ----------------
_NC_CACHE = None


def _get_nc():
    global _NC_CACHE
    if _NC_CACHE is None:
        _NC_CACHE = build_nc()
    return _NC_CACHE


def prep_shared(W_ae, b_ae, W_c, b_c, W_s, b_s, W_h, b_h, W_al, b_al, W_o, b_o):
    bf = ml_dtypes.bfloat16
    f8 = ml_dtypes.float8_e4m3

    def wt(w, nt):  # [p, t, n] = w.T[128*t + p, n]
        wT = np.ascontiguousarray(np.asarray(w, np.float32).T)
        return np.ascontiguousarray(
            wT.reshape(nt, 128, wT.shape[1]).transpose(1, 0, 2)).astype(bf)

    def bt(b, nt):  # [p, t] = b[128*t + p]
        return np.ascontiguousarray(
            np.asarray(b, np.float32).reshape(nt, 128).T).astype(np.float32)

    def rep(v):  # [128, len(v)] f32 replicated rows
        return np.ascontiguousarray(
            np.tile(np.asarray(v, np.float32)[None, :], (128, 1)))

    # w_ae_dr[p, gt, i, r] = (W_ae*WSCALE).T[f, r], f = gt*256 + 2p + i
    waeT = (np.asarray(W_ae, np.float32) * WSCALE).T.astype(f8)  # [F, R]
    w_ae_dr = np.ascontiguousarray(
        waeT.reshape(GT, 128, 2, R).transpose(1, 0, 2, 3))

    return {
        "w_ae_dr": w_ae_dr,
        "w_c_t": wt(W_c, RT),
        "w_s_t": wt(W_s, RT),
        "w_h_t": wt(W_h, RT),
        "w_o_t": wt(W_o, RT),
        "wal_rep": rep(np.asarray(W_al, np.float32)[0]).astype(bf),
        "b_ae": bt(b_ae, RT),
        "bcbh": rep(np.asarray(b_c, np.float32) + np.asarray(b_h, np.float32)),
        "bsbh": rep(np.asarray(b_s, np.float32) + np.asarray(b_h, np.float32)),
        "b_o_bcast": rep(b_o),
        "ident": np.eye(128, dtype=bf),
        "ident4": np.ascontiguousarray(
            np.broadcast_to(np.eye(128, dtype=bf)[:, None, :],
                            (128, S_PER_CHUNK, 128))),
    }


def make_in_maps(h, sentinel, att_feats, shared):
    h = np.asarray(h, np.float32)
    sentinel = np.asarray(sentinel, np.float32)
    att_feats = np.asarray(att_feats, np.float32)
    in_maps = []
    for i in range(NCORES):
        sl = slice(i * B_LOC, (i + 1) * B_LOC)
        m = dict(shared)
        m["h"] = np.ascontiguousarray(h[sl])
        m["sentinel"] = np.ascontiguousarray(sentinel[sl])
        m["att_feats"] = np.ascontiguousarray(att_feats[sl])
        in_maps.append(m)
    return in_maps


def kernel(h, sentinel, att_feats, W_ae, b_ae, W_c, b_c, W_s, b_s,
           W_h, b_h, W_al, b_al, W_o, b_o):
    shared = prep_shared(W_ae, b_ae, W_c, b_c, W_s, b_s, W_h, b_h, W_al, b_al, W_o, b_o)
    in_maps = make_in_maps(h, sentinel, att_feats, shared)
    nc = _get_nc()
    from concourse.bass_utils import run_bass_kernel_spmd
    res = run_bass_kernel_spmd(nc, in_maps, core_ids=list(range(NCORES)))
    out = np.concatenate([res.results[i]["out"] for i in range(NCORES)], axis=0)
    return np.ascontiguousarray(out.astype(np.float32))


if __name__ == "__main__":
    build_nc()
    print("built ok")


# revision 16
# speedup vs baseline: 1.2052x; 1.2052x over previous
"""AdaAttention Trainium2 kernel — data-parallel over batch across 8 NeuronCores.

Full shapes: h [1024,512], sentinel [1024,512], att_feats [1024,96,2048] -> out [1024,512].
Per core: b=128 batch rows; 24 chunks of 4 slots (512 tokens).

v4 pipeline (fp8 DoubleRow MM1, natural-layout MM2, DVE logits, streaming flash cHat):
  out = tanh(cHat @ W_oT + h @ W_oT + b_o), cHat = sum_s alpha_s * img_all_s.
  C = sum_s exp(l_s) * (img_all_s @ W_oT) accumulates in one persistent PSUM bank
  (no max subtraction: |logits| <~ 12, exp safe in f32); divide by d = sum exp at end.

Per chunk:
  att_feats --1 SWDGE DMA, f32->fp8e4 cast--> nat[b,4,2048]
  4x xbar (u16 view) -> attf[p=g%128, gt, i_s, b]  (g = f//2: fp8 byte pairs)
  MM1 (DoubleRow fp8, W_ae x256): attT[r,x] = relu(psum/256 + b_ae)  bf16
  MM2' (x4 slots): psum[b,a] = attT_i.T @ W_cT (stationary attT blocks)
  hA = tanh(psum + h_e + b_c + b_h) [DVE add + ACT tanh]
  logit col = DVE mult+reduce(hA * wal_rep)   (no PE matmul for logits)
  Z_i[b,o] = attT_i.T @ W_oT -> bf16 ; exp (ACT) -> e_sb[:, 1+4c..]
  flash (deferred 1 chunk): C += diag(e_t) @ Z_t  (diag built on GpSimd)
Final: out = tanh(C/d + h@W_oT + b_o). b_al skipped (softmax shift-invariant).
Startup: chunk-0 cast DMA issued first; prep split so the sentinel cross-engine
chain never head-blocks mm1(0) in the PE FIFO.
"""
import sys

for p in ("/opt/trn_rl_repo", "/opt/pypackages"):
    if p not in sys.path:
        sys.path.insert(0, p)

import numpy as np
import ml_dtypes
from contextlib import ExitStack

import concourse.bass as bass
import concourse.bacc as bacc
import concourse.mybir as mybir
from concourse import tile

F32 = mybir.dt.float32
BF16 = mybir.dt.bfloat16
FP8 = mybir.dt.float8e4
U16 = mybir.dt.uint16
AF = mybir.ActivationFunctionType
ALU = mybir.AluOpType
DR = mybir.MatmulPerfMode.DoubleRow

NCORES = 8
B_LOC = 128          # batch rows per core
S = 96               # attention slots
F = 2048             # att feature size
R = 512              # rnn size
A = 512              # att hidden size
XCHUNK = 512         # tokens per pipeline chunk (4 s-tiles)
NCHUNKS = (B_LOC * S) // XCHUNK   # 24
S_PER_CHUNK = XCHUNK // B_LOC     # 4
GT = F // 256        # 8 double-row f-tiles (256 f's each)
RT = R // 128        # 4
AT = A // 128        # 4
WSCALE = 256.0       # fp8 weight scale for W_ae


def build_nc():
    nc = bacc.Bacc("TRN2", target_bir_lowering=False, debug=False)

    # ---- DRAM parameters (per-core shard shapes) ----
    att_feats = nc.declare_dram_parameter("att_feats", [B_LOC, S, F], F32, isOutput=False)
    h_in = nc.declare_dram_parameter("h", [B_LOC, R], F32, isOutput=False)
    sent_in = nc.declare_dram_parameter("sentinel", [B_LOC, R], F32, isOutput=False)
    # w_ae_dr[p, gt, i, r] = (W_ae*256).T[f, r], f = 2*(gt*128+p)+i   (fp8)
    w_ae_d = nc.declare_dram_parameter("w_ae_dr", [128, GT, 2, R], FP8, isOutput=False)
    w_c_t = nc.declare_dram_parameter("w_c_t", [128, RT, A], BF16, isOutput=False)
    w_s_t = nc.declare_dram_parameter("w_s_t", [128, RT, A], BF16, isOutput=False)
    w_h_t = nc.declare_dram_parameter("w_h_t", [128, RT, A], BF16, isOutput=False)
    w_o_t = nc.declare_dram_parameter("w_o_t", [128, RT, R], BF16, isOutput=False)
    wal_rep_d = nc.declare_dram_parameter("wal_rep", [128, A], BF16, isOutput=False)
    b_ae_d = nc.declare_dram_parameter("b_ae", [128, RT], F32, isOutput=False)
    bcbh_d = nc.declare_dram_parameter("bcbh", [128, A], F32, isOutput=False)   # b_c + b_h
    bsbh_d = nc.declare_dram_parameter("bsbh", [128, A], F32, isOutput=False)   # b_s + b_h
    b_o_bc_d = nc.declare_dram_parameter("b_o_bcast", [128, R], F32, isOutput=False)
    ident_d = nc.declare_dram_parameter("ident", [128, 128], BF16, isOutput=False)
    ident4_d = nc.declare_dram_parameter("ident4", [128, S_PER_CHUNK, 128], BF16, isOutput=False)
    out_d = nc.declare_dram_parameter("out", [B_LOC, R], F32, isOutput=True)

    with tile.TileContext(nc) as tc, ExitStack() as ctx:
        # ---- pools ----
        cp = ctx.enter_context(tc.tile_pool(name="consts", bufs=1))
        nat_p = ctx.enter_context(tc.tile_pool(name="nat", bufs=3))
        stg_p = ctx.enter_context(tc.tile_pool(name="stg", bufs=3))
        attf_p = ctx.enter_context(tc.tile_pool(name="attf", bufs=2))
        attT_p = ctx.enter_context(tc.tile_pool(name="attT", bufs=3))
        hat_p = ctx.enter_context(tc.tile_pool(name="hat", bufs=6))
        z_p = ctx.enter_context(tc.tile_pool(name="zt", bufs=3))
        small_p = ctx.enter_context(tc.tile_pool(name="small", bufs=6))
        msel_p = ctx.enter_context(tc.tile_pool(name="msel", bufs=4))
        lcol_p = ctx.enter_context(tc.tile_pool(name="lcol", bufs=3))
        soft_p = ctx.enter_context(tc.tile_pool(name="soft", bufs=1))
        ps_mm1 = ctx.enter_context(tc.tile_pool(name="ps_mm1", bufs=2, space="PSUM"))
        ps_mm2 = ctx.enter_context(tc.tile_pool(name="ps_mm2", bufs=3, space="PSUM"))
        ps_z = ctx.enter_context(tc.tile_pool(name="ps_z", bufs=2, space="PSUM"))
        ps_chat = ctx.enter_context(tc.tile_pool(name="ps_chat", bufs=1, space="PSUM"))

        nat_tiles = {}

        stg_tiles = {}

        def stage_load(c):
            s0 = c * S_PER_CHUNK
            nat = nat_p.tile([B_LOC, S_PER_CHUNK, F], FP8, tag="nat", name=f"nat_{c}")
            nc.gpsimd.dma_start(out=nat[:, 0:2, :], in_=att_feats[:, s0:s0 + 2, :])
            stg = stg_p.tile([B_LOC, 2, F], F32, tag="stg", name=f"stg_{c}")
            nc.scalar.dma_start(out=stg[:], in_=att_feats[:, s0 + 2:s0 + 4, :])
            nat_tiles[c] = nat
            stg_tiles[c] = stg

        def stage_cast(c):
            nat = nat_tiles[c]
            stg = stg_tiles.pop(c)
            nc.vector.tensor_copy(nat[:, 2, :], stg[:, 0, :])
            nc.scalar.activation(nat[:, 3, :], stg[:, 1, :], AF.Copy)

        def const_tile(name, shape, dtype, src):
            t = cp.tile(shape, dtype, tag=name, name=name)
            nc.scalar.dma_start(out=t[:], in_=src[:])
            return t

        # chunk 0 feed first; h / sentinel casts next (tiny; unblock prep)
        stage_load(0)
        h_bf = cp.tile([B_LOC, R], BF16, tag="h_bf", name="h_bf")
        nc.gpsimd.dma_start(out=h_bf[:], in_=h_in[:])
        sent_bf = cp.tile([B_LOC, R], BF16, tag="sent_bf", name="sent_bf")
        nc.gpsimd.dma_start(out=sent_bf[:], in_=sent_in[:])

        # ---- constants (mm1 deps first, then prep deps, then stage_b1 deps) ----
        ident = const_tile("ident", [128, 128], BF16, ident_d)
        ident4 = const_tile("ident4", [128, S_PER_CHUNK, 128], BF16, ident4_d)
        w_ae = const_tile("w_ae", [128, GT, 2, R], FP8, w_ae_d)
        b_ae = const_tile("b_ae", [128, RT], F32, b_ae_d)
        w_h = const_tile("w_h", [128, RT, A], BF16, w_h_t)
        w_s = const_tile("w_s", [128, RT, A], BF16, w_s_t)
        w_o = const_tile("w_o", [128, RT, R], BF16, w_o_t)
        wal_rep = const_tile("wal_rep", [128, A], BF16, wal_rep_d)
        bcbh = const_tile("bcbh", [128, A], F32, bcbh_d)
        bsbh = const_tile("bsbh", [128, A], F32, bsbh_d)
        b_o_bc = const_tile("b_o_bc", [128, R], F32, b_o_bc_d)
        w_c = const_tile("w_c", [128, RT, A], BF16, w_c_t)

        stage_load(1)

        # e_sb[:, t] = exp(logit_t), t=0 sentinel, t=1.. att slots
        e_sb = cp.tile([B_LOC, 1 + S], F32, tag="e_sb", name="e_sb")
        prep_out = {}

        def prep_a():
            """PE-heavy prep + sentinel logit chain (no PE work after the chain)."""
            hT = cp.tile([128, RT, B_LOC], BF16, tag="hT", name="hT")
            sentT = cp.tile([128, RT, B_LOC], BF16, tag="sentT", name="sentT")
            for rb in range(RT):
                pt = ps_mm2.tile([128, 1024], BF16, tag="mm2", name=f"pt_h{rb}")
                nc.tensor.transpose(pt[:, :128], h_bf[:, rb * 128:(rb + 1) * 128], ident[:])
                nc.vector.tensor_copy(hT[:, rb, :], pt[:, :128])
                pt2 = ps_mm2.tile([128, 1024], BF16, tag="mm2", name=f"pt_s{rb}")
                nc.tensor.transpose(pt2[:, :128], sent_bf[:, rb * 128:(rb + 1) * 128], ident[:])
                nc.vector.tensor_copy(sentT[:, rb, :], pt2[:, :128])

            # h_e (natural [b, a]) -> he_c = h_e + b_c + b_h ; he_s = h_e + b_s + b_h
            ps_he = ps_mm2.tile([128, A], F32, tag="mm2", name="ps_he")
            for rb in range(RT):
                nc.tensor.matmul(ps_he[:], hT[:, rb, :], w_h[:, rb, :],
                                 start=(rb == 0), stop=(rb == RT - 1))
            he_c = cp.tile([B_LOC, A], F32, tag="he_c", name="he_c")
            nc.vector.tensor_tensor(out=he_c[:], in0=ps_he[:], in1=bcbh[:], op=ALU.add)
            he_s = cp.tile([B_LOC, A], F32, tag="he_s", name="he_s")
            nc.vector.tensor_tensor(out=he_s[:], in0=ps_he[:], in1=bsbh[:], op=ALU.add)

            # sentinel embed + Zsent + H_o  (PE)
            ps_se = ps_mm2.tile([128, A], F32, tag="mm2", name="ps_se")
            for rb in range(RT):
                nc.tensor.matmul(ps_se[:], sentT[:, rb, :], w_s[:, rb, :],
                                 start=(rb == 0), stop=(rb == RT - 1))
            ps_zs = ps_z.tile([128, R], F32, tag="z", name="ps_zs")
            for rb in range(RT):
                nc.tensor.matmul(ps_zs[:], sentT[:, rb, :], w_o[:, rb, :],
                                 start=(rb == 0), stop=(rb == RT - 1))
            zs_sb = cp.tile([B_LOC, R], BF16, tag="zs_sb", name="zs_sb")
            nc.scalar.activation(zs_sb[:], ps_zs[:], AF.Copy)
            ps_ho = ps_z.tile([128, R], F32, tag="z", name="ps_ho")
            for rb in range(RT):
                nc.tensor.matmul(ps_ho[:], hT[:, rb, :], w_o[:, rb, :],
                                 start=(rb == 0), stop=(rb == RT - 1))
            h_o = cp.tile([B_LOC, R], F32, tag="h_o", name="h_o")
            nc.vector.tensor_tensor(out=h_o[:], in0=ps_ho[:], in1=b_o_bc[:], op=ALU.add)

            # sentinel logit chain (DVE/ACT only)
            pre0 = small_p.tile([B_LOC, A], BF16, tag="hatmp", name="pre0")
            nc.vector.tensor_tensor(out=pre0[:], in0=ps_se[:], in1=he_s[:], op=ALU.add)
            hA0 = hat_p.tile([B_LOC, A], BF16, tag="hat", name="hA0")
            nc.scalar.activation(hA0[:], pre0[:], AF.Tanh)
            ttr0 = small_p.tile([B_LOC, A], BF16, tag="ttro", name="ttr0")
            lc0 = lcol_p.tile([B_LOC, 1], F32, tag="lc", name="lc0")
            nc.vector.tensor_tensor(out=ttr0[:], in0=hA0[:], in1=wal_rep[:], op=ALU.mult)
            nc.vector.tensor_reduce(out=lc0[:], in_=ttr0[:], op=ALU.add,
                                    axis=mybir.AxisListType.X)
            nc.scalar.activation(e_sb[:, 0:1], lc0[:], AF.Exp)
            prep_out.update(h_o=h_o, he_c=he_c, zs_sb=zs_sb)

        def prep_b():
            """Open the persistent cHat accumulation with the sentinel term."""
            ps_cH = ps_chat.tile([B_LOC, R], F32, name="ps_cH")
            ms0 = msel_p.tile([128, 128], BF16, tag="msel", name="ms0")
            nc.vector.tensor_scalar(out=ms0[:], in0=ident[:], scalar1=e_sb[:, 0:1],
                                    scalar2=None, op0=ALU.mult)
            nc.tensor.matmul(ps_cH[:], ms0[:], prep_out["zs_sb"][:], start=True,
                             stop=False, skip_group_check=True)
            prep_out.update(ps_cH=ps_cH)

        # ---- main pipeline stages ----
        attT_chunks = {}
        z_chunks = {}

        def stage_mm1(c):
            nat = nat_tiles.pop(c)
            # 4 per-slot xbar transposes (u16 = fp8 byte pair): attf[p, gt, i_s, b]
            attf = attf_p.tile([128, GT, S_PER_CHUNK, 128], U16, tag="attf", name=f"attf_{c}")
            for i in range(S_PER_CHUNK):
                eng = nc.sync if i < 2 else nc.scalar
                eng.dma_start(out=attf[:, :, i, :], in_=nat[:, i, :].bitcast(U16),
                              transpose=True)
            attT = attT_p.tile([128, RT, XCHUNK], BF16, tag="attT", name=f"attT_{c}")
            for rb in range(RT):
                ps1 = ps_mm1.tile([128, XCHUNK], F32, tag="mm1", name=f"ps1_{c}_{rb}")
                for gt in range(GT):
                    rhs = attf[:, gt].bitcast(FP8).rearrange("p s (n two) -> p two s n", two=2)
                    nc.tensor.matmul(ps1[:], w_ae[:, gt, :, rb * 128:(rb + 1) * 128],
                                     rhs, start=(gt == 0), stop=(gt == GT - 1),
                                     perf_mode=DR)
                nc.scalar.activation(attT[:, rb, :], ps1[:], AF.Relu,
                                     bias=b_ae[:, rb:rb + 1], scale=1.0 / WSCALE)
            attT_chunks[c] = attT

        def stage_b1(c):
            attT = attT_chunks.pop(c)
            he_c = prep_out["he_c"]
            zt = z_p.tile([128, S_PER_CHUNK, R], BF16, tag="zt", name=f"zt_{c}")
            lcol = lcol_p.tile([B_LOC, S_PER_CHUNK], F32, tag="lc", name=f"lcol_{c}")
            # PE: all MM2 groups first
            ps2s = []
            for i in range(S_PER_CHUNK):
                ps2 = ps_mm2.tile([128, A], F32, tag="mm2", name=f"ps2_{c}_{i}")
                for rb in range(RT):
                    nc.tensor.matmul(ps2[:], attT[:, rb, i * 128:(i + 1) * 128],
                                     w_c[:, rb, :], start=(rb == 0), stop=(rb == RT - 1))
                ps2s.append(ps2)
            # DVE adds chase the MM2 groups; ACT tanh chases the adds
            tmps = []
            for i in range(S_PER_CHUNK):
                tmp = small_p.tile([B_LOC, A], BF16, tag="hatmp", name=f"hatmp_{c}_{i}")
                nc.vector.tensor_tensor(out=tmp[:], in0=ps2s[i][:], in1=he_c[:], op=ALU.add)
                tmps.append(tmp)
            hts = []
            for i in range(S_PER_CHUNK):
                ht = hat_p.tile([B_LOC, A], BF16, tag="hat", name=f"hat_{c}_{i}")
                nc.scalar.activation(ht[:], tmps[i][:], AF.Tanh)
                hts.append(ht)
            # PE: Z groups
            pszs = []
            for i in range(S_PER_CHUNK):
                psz = ps_z.tile([128, R], F32, tag="z", name=f"psz_{c}_{i}")
                for rb in range(RT):
                    nc.tensor.matmul(psz[:], attT[:, rb, i * 128:(i + 1) * 128],
                                     w_o[:, rb, :], start=(rb == 0), stop=(rb == RT - 1))
                pszs.append(psz)
            # Z copies split DVE/ACT
            for i in range(S_PER_CHUNK):
                if i % 2 == 0:
                    nc.vector.tensor_copy(zt[:, i, :], pszs[i][:])
                else:
                    nc.scalar.activation(zt[:, i, :], pszs[i][:], AF.Copy)
            # logits: DVE mult + reduce per slot, then one exp (ACT)
            for i in range(S_PER_CHUNK):
                ttro = small_p.tile([B_LOC, A], BF16, tag="ttro", name=f"ttro_{c}_{i}")
                nc.vector.tensor_tensor(out=ttro[:], in0=hts[i][:], in1=wal_rep[:],
                                        op=ALU.mult)
                nc.vector.tensor_reduce(out=lcol[:, i:i + 1], in_=ttro[:], op=ALU.add,
                                        axis=mybir.AxisListType.X)
            z_chunks[c] = zt
            nc.scalar.activation(
                e_sb[:, 1 + c * S_PER_CHUNK: 1 + (c + 1) * S_PER_CHUNK], lcol[:], AF.Exp)

        def stage_flash(c):
            ps_cH = prep_out["ps_cH"]
            zt = z_chunks.pop(c)
            ms4 = msel_p.tile([128, S_PER_CHUNK, 128], BF16, tag="msel", name=f"ms4_{c}")
            nc.vector.tensor_tensor(
                out=ms4[:], in0=ident4[:],
                in1=e_sb[:, 1 + c * S_PER_CHUNK: 1 + (c + 1) * S_PER_CHUNK]
                    .unsqueeze(2).broadcast_to([128, S_PER_CHUNK, 128]),
                op=ALU.mult)
            for i in range(S_PER_CHUNK):
                t = c * S_PER_CHUNK + i
                nc.tensor.matmul(ps_cH[:], ms4[:, i, :], zt[:, i, :],
                                 start=False, stop=(t == S - 1), skip_group_check=True)

        # ---- build pipeline ----
        stage_cast(0)
        stage_load(2)
        stage_mm1(0)
        prep_a()
        stage_cast(1)
        stage_load(3)
        stage_mm1(1)
        prep_b()
        stage_cast(2)
        for c in range(2, NCHUNKS + 5):
            if 3 <= c <= NCHUNKS + 2:
                stage_flash(c - 3)
            if 2 <= c <= NCHUNKS + 1:
                stage_b1(c - 2)
            if c < NCHUNKS:
                stage_mm1(c)
            if c + 2 < NCHUNKS:
                stage_load(c + 2)
            if c + 1 < NCHUNKS:
                stage_cast(c + 1)

        # ---- final: out = tanh(C/d + H_o) ----
        ps_cH = prep_out["ps_cH"]
        h_o = prep_out["h_o"]
        dsum = soft_p.tile([B_LOC, 1], F32, tag="soft", name="dsum")
        nc.vector.tensor_reduce(out=dsum[:], in_=e_sb[:], op=ALU.add,
                                axis=mybir.AxisListType.X)
        rin = soft_p.tile([B_LOC, 1], F32, tag="rin", name="rin")
        nc.vector.reciprocal(rin[:], dsum[:])
        chn = soft_p.tile([B_LOC, R], F32, tag="chn", name="chn")
        nc.vector.tensor_scalar(out=chn[:], in0=ps_cH[:], scalar1=rin[:],
                                scalar2=None, op0=ALU.mult)
        pre = soft_p.tile([B_LOC, R], F32, tag="pre", name="pre")
        nc.vector.tensor_tensor(out=pre[:], in0=chn[:], in1=h_o[:], op=ALU.add)
        out_sb = soft_p.tile([B_LOC, R], F32, tag="out_sb", name="out_sb")
        nc.scalar.activation(out_sb[:], pre[:], AF.Tanh)
        nc.sync.dma_start(out=out_d[:], in_=out_sb[:])

    nc.compile()
    return nc


# ---------------- host side ----------------
_NC_CACHE = None


def _get_nc():
    global _NC_CACHE
    if _NC_CACHE is None:
        _NC_CACHE = build_nc()
    return _NC_CACHE


def prep_shared(W_ae, b_ae, W_c, b_c, W_s, b_s, W_h, b_h, W_al, b_al, W_o, b_o):
    bf = ml_dtypes.bfloat16
    f8 = ml_dtypes.float8_e4m3

    def wt(w, nt):  # [p, t, n] = w.T[128*t + p, n]
        wT = np.ascontiguousarray(np.asarray(w, np.float32).T)
        return np.ascontiguousarray(
            wT.reshape(nt, 128, wT.shape[1]).transpose(1, 0, 2)).astype(bf)

    def bt(b, nt):  # [p, t] = b[128*t + p]
        return np.ascontiguousarray(
            np.asarray(b, np.float32).reshape(nt, 128).T).astype(np.float32)

    def rep(v):  # [128, len(v)] f32 replicated rows
        return np.ascontiguousarray(
            np.tile(np.asarray(v, np.float32)[None, :], (128, 1)))

    # w_ae_dr[p, gt, i, r] = (W_ae*WSCALE).T[f, r], f = gt*256 + 2p + i
    waeT = (np.asarray(W_ae, np.float32) * WSCALE).T.astype(f8)  # [F, R]
    w_ae_dr = np.ascontiguousarray(
        waeT.reshape(GT, 128, 2, R).transpose(1, 0, 2, 3))

    return {
        "w_ae_dr": w_ae_dr,
        "w_c_t": wt(W_c, RT),
        "w_s_t": wt(W_s, RT),
        "w_h_t": wt(W_h, RT),
        "w_o_t": wt(W_o, RT),
        "wal_rep": rep(np.asarray(W_al, np.float32)[0]).astype(bf),
        "b_ae": bt(b_ae, RT),
        "bcbh": rep(np.asarray(b_c, np.float32) + np.asarray(b_h, np.float32)),
        "bsbh": rep(np.asarray(b_s, np.float32) + np.asarray(b_h, np.float32)),
        "b_o_bcast": rep(b_o),
        "ident": np.eye(128, dtype=bf),
        "ident4": np.ascontiguousarray(
            np.broadcast_to(np.eye(128, dtype=bf)[:, None, :],
                            (128, S_PER_CHUNK, 128))),
    }


def make_in_maps(h, sentinel, att_feats, shared):
    h = np.asarray(h, np.float32)
    sentinel = np.asarray(sentinel, np.float32)
    att_feats = np.asarray(att_feats, np.float32)
    in_maps = []
    for i in range(NCORES):
        sl = slice(i * B_LOC, (i + 1) * B_LOC)
        m = dict(shared)
        m["h"] = np.ascontiguousarray(h[sl])
        m["sentinel"] = np.ascontiguousarray(sentinel[sl])
        m["att_feats"] = np.ascontiguousarray(att_feats[sl])
        in_maps.append(m)
    return in_maps


def kernel(h, sentinel, att_feats, W_ae, b_ae, W_c, b_c, W_s, b_s,
           W_h, b_h, W_al, b_al, W_o, b_o):
    shared = prep_shared(W_ae, b_ae, W_c, b_c, W_s, b_s, W_h, b_h, W_al, b_al, W_o, b_o)
    in_maps = make_in_maps(h, sentinel, att_feats, shared)
    nc = _get_nc()
    from concourse.bass_utils import run_bass_kernel_spmd
    res = run_bass_kernel_spmd(nc, in_maps, core_ids=list(range(NCORES)))
    out = np.concatenate([res.results[i]["out"] for i in range(NCORES)], axis=0)
    return np.ascontiguousarray(out.astype(np.float32))


if __name__ == "__main__":
    build_nc()
    print("built ok")
